# revision 1
# baseline (speedup 1.0000x reference)
"""EEND-SS loss device kernel (raw Bass, SPMD over 8 cores).

Device computes, per core (B_LOC=4 samples):
  - "gram":  Gram matrix of [sep rows(12) | src rows(12) | ones] over T,
             chunk-blocked so host extracts all pairwise dots / sums / sq-sums.
  - "dgram": Gram of [logp | log1mp] x [tgt | 1-tgt] over T_sub=1000 for the
             diarization BCE (labels nearest-neighbor subsampled on the fly
             via strided DMA).
Host does the tiny O(B) permutation-invariant (PIT) math + existence BCE.

Pipeline per phase (T split into NPH column-phases of W chunks each):
  DMA (row-major, full-rate)  ->  staging[ph%3]   [128, 24*W]
  repack (DVE/ACT/GPSIMD 8 rows each) -> blocked[ph%4]  [128, 26*W]
  PE matmuls on blocked (lhsT must be a single contiguous free dim)

Blocked layout: free = (g, r, c): index = 130*g + 5*r + c, r in 0..25
 (rows 0..23 = data row r = s*6 + t*3 + i, row 24 = ones, row 25 = pad),
 c in 0..4, chunk = 5*g + c.  Partition p holds T positions
 [TCOLS*p, TCOLS*(p+1)), chunk n is column n of that view.

  scheme "fp32": per block g one matmul, lhsT = rhs = blocked[:, 130g:130g+125]
      psum_gram[125,125] += lhsT.T @ rhs ; m = 5*r + c
      host: Gamma[ra, rb] = sum_c psum[5*ra+c, 5*rb+c]

  scheme "f32r": per supergroup g2 (blocks 2g2, 2g2+1), two float32r matmuls
      with N=260 >= 256 for 1 cycle/row:
      psum_a += blocked[:, 260g2      : 260g2+125].T @ rhs(26 rows x 10 chunks)
      psum_b += blocked[:, 260g2+130 : 260g2+255].T @ rhs
      rhs free dims ordered (r, h, c): n = r*10 + 5h + c
"""

import numpy as np
from contextlib import ExitStack

import concourse.bass as bass
from concourse import mybir

F32 = mybir.dt.float32
F32R = mybir.dt.float32r
BF16 = mybir.dt.bfloat16
AFT = mybir.ActivationFunctionType

C = 3
B_LOC = 4
P = 128
PD = 125           # diar partition count
TSUB = 1000
KSUB = TSUB // PD  # 8 t_sub positions per partition
NROW = 26          # 24 data rows + ones(24) + pad(25)
NDATA = 24
BLK = 5 * NROW     # 130: free elems per chunk-group block
N_STAGE = 4
N_BLOCK = 5


def build_nc(T=128000, NPH=8, scheme="fp32", TF_STRIDE=8, dual_ring=True):
    """Build the per-core Bass program. Returns (nc, meta)."""
    TCOLS = T // P
    assert TCOLS * P == T
    W = TCOLS // NPH
    assert W * NPH == TCOLS
    if scheme in ("fp32", "bf16"):
        assert W % 5 == 0
    else:
        assert W % 10 == 0
    blk_dt = BF16 if scheme == "bf16" else F32
    G = W // 5          # blocks per phase
    TF = TSUB * TF_STRIDE
    n_stage = min(N_STAGE, NPH)
    n_block = min(N_BLOCK, NPH)

    nc = bass.Bass(trn_type="TRN2", target_bir_lowering=False, debug=False)

    sep = nc.dram_tensor("sep", [B_LOC, C, T], F32, kind="ExternalInput").ap()
    src = nc.dram_tensor("src", [B_LOC, C, T], F32, kind="ExternalInput").ap()
    diar = nc.dram_tensor("diar", [B_LOC, TSUB, C], F32, kind="ExternalInput").ap()
    lab = nc.dram_tensor("lab", [B_LOC, TF, C], F32, kind="ExternalInput").ap()

    if scheme in ("fp32", "bf16"):
        gram_out = nc.dram_tensor("gram", [PD, PD + 27], F32, kind="ExternalOutput").ap()
    else:
        gram_a_out = nc.dram_tensor("gram_a", [PD, 260], F32, kind="ExternalOutput").ap()
        gram_b_out = nc.dram_tensor("gram_b", [PD, 260], F32, kind="ExternalOutput").ap()
    if scheme == "f32r":
        dgram_out = nc.dram_tensor("dgram", [NDATA, NDATA], F32, kind="ExternalOutput").ap()

    # SBUF
    stg = [nc.alloc_sbuf_tensor(f"stg{i}", [P, NDATA * W], F32).ap()
           for i in range(n_stage)]
    blk = [nc.alloc_sbuf_tensor(f"blk{i}", [P, NROW * W], blk_dt).ap()
           for i in range(n_block)]
    p0 = nc.alloc_sbuf_tensor("p0", [PD, B_LOC * KSUB * C], F32).ap()   # (s,k,j)
    lf = nc.alloc_sbuf_tensor("lf", [PD, B_LOC * (TF // PD) * C], F32).ap()  # full labels
    # ll/rr layout: free = (k, q, s, j): k-slice contiguous 24 for matmul lhsT
    ll = nc.alloc_sbuf_tensor("ll", [PD, KSUB * 2 * B_LOC * C], F32).ap()
    rr = nc.alloc_sbuf_tensor("rr", [PD, KSUB * 2 * B_LOC * C], F32).ap()
    if scheme in ("fp32", "bf16"):
        out_sb = nc.alloc_sbuf_tensor("out_sb", [PD, PD + 27], F32).ap()
    else:
        out_sb_a = nc.alloc_sbuf_tensor("out_sb_a", [PD, 260], F32).ap()
        out_sb_b = nc.alloc_sbuf_tensor("out_sb_b", [PD, 260], F32).ap()
    d_sb = nc.alloc_sbuf_tensor("d_sb", [NDATA, NDATA], F32).ap()

    # PSUM
    if scheme in ("fp32", "bf16"):
        ps_g = nc.alloc_psum_tensor("ps_g", [PD, PD], F32).ap()
    else:
        ps_a = nc.alloc_psum_tensor("ps_a", [PD, 260], F32).ap()
        ps_b = nc.alloc_psum_tensor("ps_b", [PD, 260], F32).ap()
    ps_d = nc.alloc_psum_tensor("ps_d", [NDATA, NDATA], F32).ap()

    # helper views
    def stg3(i):
        return stg[i].rearrange("p (r n) -> p r n", r=NDATA)

    def stg4(i):  # [p, r, g, c]
        return stg[i].rearrange("p (r g c) -> p r g c", r=NDATA, g=G)

    def blk4(i):  # [p, r, g, c] view of blocked (g, r, c) layout
        return blk[i].rearrange("p (g r c) -> p r g c", g=G, r=NROW)

    # repack row split across the three copy engines
    ROWS = {"dve": (0, 17), "act": (17, 24)}

    with ExitStack() as ctx:
        st_sems = [ctx.enter_context(nc.semaphore(f"st_sem{i}")) for i in range(NPH)]
        pdma_sem = ctx.enter_context(nc.semaphore("pdma_sem"))
        ldma_sem = ctx.enter_context(nc.semaphore("ldma_sem"))
        odma_sem = ctx.enter_context(nc.semaphore("odma_sem"))
        rpk_dve = ctx.enter_context(nc.semaphore("rpk_dve"))
        rpk_act = ctx.enter_context(nc.semaphore("rpk_act"))
        rpk_gp = ctx.enter_context(nc.semaphore("rpk_gp"))
        act_sem = ctx.enter_context(nc.semaphore("act_sem"))
        dve_sem = ctx.enter_context(nc.semaphore("dve_sem"))
        pe_sem = ctx.enter_context(nc.semaphore("pe_sem"))
        block = ctx.enter_context(nc.Block())

        def issue_phase_dmas(eng, ph, samples):
            s3 = stg3(ph % n_stage)
            for s in samples:
                for t, big in enumerate((sep, src)):
                    r0 = s * 6 + t * 3
                    src_ap = big[s].rearrange("i (p n) -> p i n", p=P)[
                        :, :, ph * W:(ph + 1) * W]
                    eng.dma_start(out=s3[:, r0:r0 + 3, :], in_=src_ap
                                  ).then_inc(st_sems[ph], 16)

        @block.sync
        def _(sync: bass.BassEngine):
            # big data split across both HWDGE rings (desc-gen is the issue-rate
            # bottleneck: ~0.8us per DMA, serialized per ring)
            for ph in range(NPH):
                if ph >= n_stage:
                    # WAR: staging slot reused; wait repack of ph - n_stage done
                    v = ph - n_stage + 1
                    sync.wait_ge(rpk_dve, v)
                    sync.wait_ge(rpk_act, v)
                issue_phase_dmas(sync, ph, (0, 1) if dual_ring else (0, 1, 2, 3))

            # outputs at the very end
            sync.wait_ge(dve_sem, 4)
            if scheme in ("fp32", "bf16"):
                sync.dma_start(out=gram_out, in_=out_sb).then_inc(odma_sem, 16)
                n_out = 1
            else:
                sync.dma_start(out=gram_a_out, in_=out_sb_a).then_inc(odma_sem, 16)
                sync.dma_start(out=gram_b_out, in_=out_sb_b).then_inc(odma_sem, 16)
                sync.dma_start(out=dgram_out, in_=d_sb).then_inc(odma_sem, 16)
                n_out = 3
            sync.wait_ge(odma_sem, 16 * n_out)

        def out_rpk(ph):
            v = blk4(ph % n_block)
            return v.bitcast(F32R) if scheme == "f32r" else v

        def repack(eng, sem, rows, copy_fn, mid_hook=None):
            r0, r1 = rows
            for ph in range(NPH):
                eng.wait_ge(st_sems[ph], 16 * 8)
                if ph >= n_block:
                    eng.wait_ge(pe_sem, ph - n_block + 1)
                copy_fn(
                    out_rpk(ph)[:, r0:r1, :, :],
                    stg4(ph % n_stage)[:, r0:r1, :, :],
                ).then_inc(sem, 1)
                if mid_hook is not None and ph == mid_hook[0]:
                    mid_hook[1]()

        @block.gpsimd
        def _(gpsimd: bass.BassEngine):
            if scheme in ("fp32", "bf16"):
                gpsimd.memset(out_sb, 0.0).then_inc(rpk_gp, 1)
            # ones(row 24) + pad(row 25) once per blocked slot; no repack here
            # (gpsimd copies measured ~6x slower than DVE)
            for i in range(n_block):
                ap1 = blk[i].rearrange("p (g x) -> p g x", g=G)[:, :, 5 * NDATA:5 * NROW]
                if scheme == "f32r":
                    ap1 = ap1.bitcast(F32R)
                gpsimd.memset(ap1, 1.0).then_inc(rpk_gp, 1)

        @block.scalar
        def _(scalar: bass.BassEngine):
            def diar_dmas():
                # diar inputs on the scalar HWDGE ring, contiguous layouts only
                # (12B-granule gathers would grind the SDMA engines for ~30us)
                scalar.dma_start(
                    out=p0.rearrange("p (s x) -> p s x", s=B_LOC),
                    in_=diar.rearrange("s (p k) j -> p s (k j)", p=PD),
                ).then_inc(pdma_sem, 16)
                scalar.dma_start(
                    out=lf.rearrange("p (s x) -> p s x", s=B_LOC),
                    in_=lab.rearrange("s (p e) j -> p s (e j)", p=PD),
                ).then_inc(ldma_sem, 16)

            if dual_ring:
                issue_phase_dmas(scalar, 0, (2, 3))
                if NPH > 1:
                    issue_phase_dmas(scalar, 1, (2, 3))
            diar_dmas()
            # per phase: repack this phase, then issue phase ph+2's DMAs
            # (ph+1 already issued) so repack(0) is never starved
            r0, r1 = ROWS["act"]

            def diar_acts():
                rrk = rr.rearrange("p (k q s j) -> p k q s j", k=KSUB, q=2, s=B_LOC)
                llk = ll.rearrange("p (k q s j) -> p k q s j", k=KSUB, q=2, s=B_LOC)
                p0k = p0.rearrange("p (s k j) -> p k s j", s=B_LOC, k=KSUB)
                scalar.wait_ge(pdma_sem, 16)
                scalar.activation(llk[:, :, 0, :, :], p0k, AFT.Ln).then_inc(act_sem, 1)
                scalar.activation(llk[:, :, 1, :, :], p0k, AFT.Ln,
                                  scale=-1.0, bias=1.0).then_inc(act_sem, 1)
                scalar.wait_ge(dve_sem, 1)
                scalar.activation(rrk[:, :, 1, :, :], rrk[:, :, 0, :, :], AFT.Copy,
                                  scale=-1.0, bias=1.0).then_inc(act_sem, 1)

            for ph in range(NPH):
                scalar.wait_ge(st_sems[ph], 16 * 8)
                if ph >= n_block:
                    scalar.wait_ge(pe_sem, ph - n_block + 1)
                scalar.activation(
                    out_rpk(ph)[:, r0:r1, :, :],
                    stg4(ph % n_stage)[:, r0:r1, :, :],
                    AFT.Copy).then_inc(rpk_act, 1)
                if ph == min(1, NPH - 1):
                    diar_acts()
                if dual_ring and ph + 2 < NPH:
                    if ph + 2 >= n_stage:
                        scalar.wait_ge(rpk_dve, ph + 2 - n_stage + 1)
                        scalar.wait_ge(rpk_act, ph + 2 - n_stage + 1)
                    issue_phase_dmas(scalar, ph + 2, (2, 3))

        @block.vector
        def _(vector: bass.BassEngine):
            def diar_dve():
                # nearest-neighbor label subsample: lf (s, 8k+f, j), f=0
                rrk = rr.rearrange("p (k q s j) -> p k q s j", k=KSUB, q=2, s=B_LOC)
                lf5 = lf.rearrange("p (s k f j) -> p k s f j", s=B_LOC, k=KSUB,
                                   f=(TF // PD) // KSUB)[:, :, :, 0, :]
                vector.wait_ge(ldma_sem, 16)
                vector.tensor_copy(rrk[:, :, 0, :, :], lf5).then_inc(dve_sem, 1)
                vector.wait_ge(act_sem, 2)
                vector.tensor_scalar_max(ll[:, :], ll[:, :], -100.0).then_inc(dve_sem, 1)

            repack(vector, rpk_dve, ROWS["dve"], vector.tensor_copy,
                   mid_hook=(min(0, NPH - 1), diar_dve))
            if scheme in ("fp32", "bf16"):
                vector.wait_ge(pe_sem, min(2, NPH - 1) + 2)
                vector.wait_ge(rpk_gp, 1)
                vector.tensor_copy(out_sb[0:NDATA, PD:PD + NDATA], ps_d
                                   ).then_inc(dve_sem, 1)
                vector.wait_ge(pe_sem, NPH + 1)
                vector.tensor_copy(out_sb[:, 0:PD], ps_g).then_inc(dve_sem, 1)
            else:
                vector.tensor_copy(out_sb_a, ps_a)
                vector.tensor_copy(out_sb_b, ps_b).then_inc(dve_sem, 1)
                vector.wait_ge(pe_sem, NPH + 1)
                vector.tensor_copy(d_sb, ps_d).then_inc(dve_sem, 1)

        @block.tensor
        def _(tensor: bass.BassEngine):
            nmm = 0
            if scheme in ("fp32", "bf16"):
                total_mm = NPH * G
            else:
                total_mm = NPH * (G // 2) * 2
            for ph in range(NPH):
                v = ph + 1
                tensor.wait_ge(rpk_dve, v)
                tensor.wait_ge(rpk_act, v)
                tensor.wait_ge(rpk_gp, n_block + (1 if scheme in ('fp32', 'bf16') else 0))
                b = blk[ph % n_block]
                if scheme in ("fp32", "bf16"):
                    for g in range(G):
                        ap = b[:, BLK * g: BLK * g + 125]
                        mm = tensor.matmul(ps_g, ap, ap,
                                           start=(nmm == 0), stop=(nmm == total_mm - 1))
                        nmm += 1
                else:
                    b5 = b.rearrange("p (G h r c) -> p G r h c", h=2, r=NROW, c=5)
                    for g2 in range(G // 2):
                        rhs = b5[:, g2].bitcast(F32R)        # [p, 26, 2, 5]
                        la = b[:, 2 * BLK * g2: 2 * BLK * g2 + 125].bitcast(F32R)
                        lb = b[:, 2 * BLK * g2 + BLK: 2 * BLK * g2 + BLK + 125].bitcast(F32R)
                        first = nmm == 0
                        last = nmm == total_mm - 2
                        mm = tensor.matmul(ps_a, la, rhs, start=first, stop=last)
                        nmm += 1
                        mm = tensor.matmul(ps_b, lb, rhs, start=first, stop=last)
                        nmm += 1
                mm.then_inc(pe_sem, 1)
                if ph == min(2, NPH - 1):
                    # diar matmuls mid-stream; lhsT k-slices contiguous 24 cols
                    tensor.wait_ge(pdma_sem, 16)
                    tensor.wait_ge(ldma_sem, 16)
                    tensor.wait_ge(act_sem, 3)
                    tensor.wait_ge(dve_sem, 2)
                    nd = 2 * B_LOC * C  # 24
                    for k in range(KSUB):
                        dmm = tensor.matmul(ps_d, ll[:, k * nd:(k + 1) * nd],
                                            rr[:, k * nd:(k + 1) * nd],
                                            start=(k == 0), stop=(k == KSUB - 1))
                    dmm.then_inc(pe_sem, 1)

    meta = dict(T=T, NPH=NPH, W=W, scheme=scheme, dual_ring=dual_ring)
    return nc, meta


# ---------------- host side ----------------

EPS = 1e-8
LAM_SISNR, LAM_DIAR, LAM_EXIST = 1.0, 0.2, 0.2
from itertools import permutations
PERMS = np.array(list(permutations(range(C))), dtype=np.int64)  # [6, 3]


def host_gamma_fp32(g125):
    """g125 [125,125] -> Gamma [25,25]; m = 5*r + c."""
    return np.einsum('acbc->ab', g125.reshape(25, 5, 25, 5).astype(np.float64))


def host_gamma_f32r(ga, gb):
    """ga/gb [125,260] -> Gamma [25,25]; m = 5*ra + ca, n = rb*10 + 5*h + cb."""
    a = ga.reshape(25, 5, 26, 2, 5).astype(np.float64)   # [ra, ca, rb, h, cb]
    b = gb.reshape(25, 5, 26, 2, 5).astype(np.float64)
    gam = np.zeros((25, 25), np.float64)
    for cc in range(5):
        gam += a[:, cc, 0:25, 0, cc]
        gam += b[:, cc, 0:25, 1, cc]
    return gam


def _clog(x):
    with np.errstate(divide='ignore'):
        return np.maximum(np.log(x), -100.0)


def host_finalize(gammas, dgrams, exist_probs, num_speakers, T=128000):
    """gammas: list of [25,25] float64 per core; dgrams list of [24,24].
    Returns the 5 scalars (np.float32)."""
    B = len(gammas) * B_LOC
    ns = np.asarray(num_speakers).astype(np.int64)

    S = np.zeros((B, C, C), np.float64)
    D = np.zeros((B, C, C), np.float64)
    for core, (gam, dg) in enumerate(zip(gammas, dgrams)):
        dg = dg.astype(np.float64)
        for s in range(B_LOC):
            b = core * B_LOC + s
            e_rows = [s * 6 + i for i in range(3)]
            t_rows = [s * 6 + 3 + j for j in range(3)]
            dot_raw = gam[np.ix_(e_rows, t_rows)]            # [i, j]
            sep_sq = np.array([gam[r, r] for r in e_rows])
            src_sq = np.array([gam[r, r] for r in t_rows])
            sum_sep = gam[e_rows, 24]
            sum_src = gam[t_rows, 24]

            dot = dot_raw - np.outer(sum_sep, sum_src) / T
            est_sq = sep_sq - sum_sep ** 2 / T               # [i]
            tgt_sq = src_sq - sum_src ** 2 / T               # [j]

            alpha = dot / (tgt_sq[None, :] + EPS)
            sig = alpha * alpha * tgt_sq[None, :] + EPS
            noise = est_sq[:, None] - 2.0 * alpha * dot + alpha * alpha * tgt_sq[None, :] + EPS
            S[b] = 10.0 * np.log10(sig / noise)

            A = dg[s * 3:s * 3 + 3, s * 3:s * 3 + 3]
            Bm = dg[12 + s * 3:12 + s * 3 + 3, 12 + s * 3:12 + s * 3 + 3]
            D[b] = -(A + Bm) / TSUB

    n_spk = np.clip(ns, 1, C)
    slot = np.arange(C)
    slot_mask = (slot[None, :] < n_spk[:, None]).astype(np.float64)
    valid = np.all((PERMS[None, :, :] < n_spk[:, None, None])
                   | (slot[None, None, :] >= n_spk[:, None, None]), axis=-1)

    S_perm = S[:, PERMS, slot]                               # [B, 6, 3]
    sisnr_mean = (S_perm * slot_mask[:, None, :]).sum(-1) / n_spk[:, None]
    sisnr_loss_p = np.where(valid, -sisnr_mean, np.inf)
    best = sisnr_loss_p.min(axis=-1)
    loss_sisnr = best.mean()
    mean_sisnr = (-best).mean()

    D_perm = D[:, PERMS, slot]
    diar_p = (D_perm * slot_mask[:, None, :]).sum(-1) / n_spk[:, None]
    loss_diar = np.where(valid, diar_p, np.inf).min(axis=-1).mean()

    ep = np.asarray(exist_probs, np.float64)
    n_ex = np.minimum(ns, C)
    ex_tgt = (np.arange(C + 1)[None, :] < n_ex[:, None]).astype(np.float64)
    bce_ex = -(ex_tgt * _clog(ep) + (1.0 - ex_tgt) * _clog(1.0 - ep))
    loss_exist = bce_ex.mean()

    total = LAM_SISNR * loss_sisnr + LAM_DIAR * loss_diar + LAM_EXIST * loss_exist
    return tuple(np.float32(v) for v in
                 (total, loss_sisnr, loss_diar, loss_exist, mean_sisnr))


def shard_inputs(separated, diar_probs, sources, labels, n_cores=8):
    maps = []
    for c in range(n_cores):
        sl = slice(B_LOC * c, B_LOC * (c + 1))
        maps.append({
            "sep": np.ascontiguousarray(separated[sl], dtype=np.float32),
            "src": np.ascontiguousarray(sources[sl], dtype=np.float32),
            "diar": np.ascontiguousarray(diar_probs[sl], dtype=np.float32),
            "lab": np.ascontiguousarray(labels[sl], dtype=np.float32),
        })
    return maps


# ---------------- kernel entry (self-contained) ----------------

N_CORES = 8
_CACHE = {}


def _get_nc():
    if "nc" not in _CACHE:
        _CACHE["nc"] = build_nc(T=128000, NPH=5, scheme="bf16", dual_ring=True)[0]
    return _CACHE["nc"]


def kernel(separated, diar_probs, exist_probs, sources, labels, num_speakers):
    """EEND-SS loss on 8 NeuronCores: batch sharded 4 samples/core; device
    computes the big time-axis Grams; host does the tiny PIT/existence math."""
    from concourse.bass_utils import run_bass_kernel_spmd

    separated = np.asarray(separated)
    diar_probs = np.asarray(diar_probs)
    exist_probs = np.asarray(exist_probs)
    sources = np.asarray(sources)
    labels = np.asarray(labels)
    num_speakers = np.asarray(num_speakers)

    nc = _get_nc()
    in_maps = shard_inputs(separated, diar_probs, sources, labels, N_CORES)
    res = run_bass_kernel_spmd(nc, in_maps, list(range(N_CORES)))

    # gram output [125, 152]: cols 0:125 = chunk-blocked Gram, cols 125:149
    # rows 0:24 carry the diar Gram
    gammas = [host_gamma_fp32(res.results[c]["gram"][:, :PD]) for c in range(N_CORES)]
    dgrams = [res.results[c]["gram"][:NDATA, PD:PD + NDATA] for c in range(N_CORES)]
    return host_finalize(gammas, dgrams, exist_probs, num_speakers, T=128000)



# revision 3
# speedup vs baseline: 1.1609x; 1.1609x over previous
"""EEND-SS loss device kernel (raw Bass, SPMD over 8 cores).

Device computes, per core (B_LOC=4 samples), the Gram matrix of
[sep rows(12) | src rows(12) | ones] over T, chunk-blocked so the host
extracts all pairwise dots / sums / sq-sums for the SI-SDR PIT loss.
sep+src are 97% of the input bytes (12.3 MB/core); the tiny diarization
BCE (diar_probs 48 KB + labels 384 KB per 4 samples) and existence BCE
are done host-side in numpy -- their scattered-granule DMAs would grind
the SDMA engines for ~30us, longer than the compute they feed.

Pipeline per phase (T split into column-phases of widths[ph] chunks each):
  DMA (sequential HBM)        ->  staging[ph%4]   [128, 24*W] f32
  repack (DVE/ACT 17/7 rows)  ->  blocked[ph%5]   [128, 26*W] bf16
  PE matmuls on blocked (lhsT must be a single contiguous free dim)

Phase ph covers the contiguous T-range [128*offs[ph], 128*(offs[ph]+W)),
split contiguously by partition: partition p holds elems
[128*offs + p*W, 128*offs + (p+1)*W).  Consecutive descriptors then read
sequential HBM addresses (full row-buffer locality, ~25 GB/s/engine)
instead of 900B-granule hops on a 4KB stride (~19 GB/s/engine).  The
Gram is invariant to this time-permutation: all 24 rows + the ones row
of a sample use the same mapping, so every product pairs equal t.

Phase widths taper (..., 50): the tail after the last input byte is
last-phase repack + matmuls + output DMA, and scales with widths[-1].
Desc-gen (DIRECT2D ~0.7-1.3us per phase-DMA, serialized per HWDGE ring)
scales with len(widths) -- 5 phases keeps it well under the DMA window.

Blocked layout: free = (g, r, c): index = 130*g + 5*r + c, r in 0..25
 (rows 0..23 = data row r = s*6 + t*3 + i, row 24 = ones, row 25 = pad),
 c in 0..4, chunk = 5*g + c.

  per block g one matmul, lhsT = rhs = blocked[:, 130g:130g+125]
      psum_gram[125,125] += lhsT.T @ rhs ; m = 5*r + c
      host: Gamma[ra, rb] = sum_c psum[5*ra+c, 5*rb+c]
"""

import numpy as np
from contextlib import ExitStack

import concourse.bass as bass
from concourse import mybir

F32 = mybir.dt.float32
BF16 = mybir.dt.bfloat16
AFT = mybir.ActivationFunctionType

C = 3
B_LOC = 4
P = 128
PD = 125           # psum partition count (25 rows x 5 chunk-cols)
NROW = 26          # 24 data rows + ones(24) + pad(25)
NDATA = 24
BLK = 5 * NROW     # 130: free elems per chunk-group block
N_STAGE = 4
N_BLOCK = 5


def build_nc(T=128000, widths=(250, 250, 250, 200, 50), dual_ring=True):
    """Build the per-core Bass program. Returns (nc, meta)."""
    TCOLS = T // P
    assert TCOLS * P == T
    widths = tuple(widths)
    NPH = len(widths)
    assert sum(widths) == TCOLS
    assert all(w % 5 == 0 for w in widths)
    offs = [sum(widths[:i]) for i in range(NPH)]
    W_MAX = max(widths)
    Gs = [w // 5 for w in widths]   # blocks per phase
    n_stage = min(N_STAGE, NPH)
    n_block = min(N_BLOCK, NPH)

    nc = bass.Bass(trn_type="TRN2", target_bir_lowering=False, debug=False)

    sep = nc.dram_tensor("sep", [B_LOC, C, T], F32, kind="ExternalInput").ap()
    src = nc.dram_tensor("src", [B_LOC, C, T], F32, kind="ExternalInput").ap()
    gram_out = nc.dram_tensor("gram", [PD, PD], F32, kind="ExternalOutput").ap()

    # SBUF
    stg = [nc.alloc_sbuf_tensor(f"stg{i}", [P, NDATA * W_MAX], F32).ap()
           for i in range(n_stage)]
    blk = [nc.alloc_sbuf_tensor(f"blk{i}", [P, NROW * W_MAX], BF16).ap()
           for i in range(n_block)]
    out_sb = nc.alloc_sbuf_tensor("out_sb", [PD, PD], F32).ap()

    # PSUM
    ps_g = nc.alloc_psum_tensor("ps_g", [PD, PD], F32).ap()

    # helper views (per-phase W: use the buffer prefix so rows stay packed)
    def stg3(i, w):
        return stg[i][:, :NDATA * w].rearrange("p (r n) -> p r n", r=NDATA)

    def stg4(i, g):  # [p, r, g, c]
        return stg[i][:, :NDATA * g * 5].rearrange(
            "p (r g c) -> p r g c", r=NDATA, g=g)

    def blk4(i, g):  # [p, r, g, c] view of blocked (g, r, c) layout
        return blk[i][:, :NROW * g * 5].rearrange(
            "p (g r c) -> p r g c", g=g, r=NROW)

    # repack row split across the two fast copy engines
    ROWS = {"dve": (0, 17), "act": (17, 24)}

    with ExitStack() as ctx:
        st_sems = [ctx.enter_context(nc.semaphore(f"st_sem{i}")) for i in range(NPH)]
        odma_sem = ctx.enter_context(nc.semaphore("odma_sem"))
        rpk_dve = ctx.enter_context(nc.semaphore("rpk_dve"))
        rpk_act = ctx.enter_context(nc.semaphore("rpk_act"))
        rpk_gp = ctx.enter_context(nc.semaphore("rpk_gp"))
        dve_sem = ctx.enter_context(nc.semaphore("dve_sem"))
        pe_sem = ctx.enter_context(nc.semaphore("pe_sem"))
        block = ctx.enter_context(nc.Block())

        def issue_phase_dmas(eng, ph, samples):
            s3 = stg3(ph % n_stage, widths[ph])
            w = widths[ph]
            for s in samples:
                for t, big in enumerate((sep, src)):
                    r0 = s * 6 + t * 3
                    src_ap = big[s][:, P * offs[ph]:P * (offs[ph] + w)
                                    ].rearrange("i (p w) -> p i w", p=P)
                    eng.dma_start(out=s3[:, r0:r0 + 3, :], in_=src_ap
                                  ).then_inc(st_sems[ph], 16)

        @block.sync
        def _(sync: bass.BassEngine):
            # big data split across both HWDGE rings (desc-gen is the issue-rate
            # bottleneck: ~0.7-1.3us per DMA, serialized per ring)
            for ph in range(NPH):
                if ph >= n_stage:
                    # WAR: staging slot reused; wait repack of ph - n_stage done
                    v = ph - n_stage + 1
                    sync.wait_ge(rpk_dve, v)
                    sync.wait_ge(rpk_act, v)
                issue_phase_dmas(sync, ph, (0, 1) if dual_ring else (0, 1, 2, 3))

            # output at the very end
            sync.wait_ge(dve_sem, 1)
            sync.dma_start(out=gram_out, in_=out_sb).then_inc(odma_sem, 16)
            sync.wait_ge(odma_sem, 16)

        @block.gpsimd
        def _(gpsimd: bass.BassEngine):
            # ones(row 24) + pad(row 25) once per blocked slot; no repack here
            # (gpsimd copies measured ~6x slower than DVE)
            for i in range(n_block):
                ap1 = blk[i].rearrange("p (g x) -> p g x", g=W_MAX // 5)[
                    :, :, 5 * NDATA:5 * NROW]
                gpsimd.memset(ap1, 1.0).then_inc(rpk_gp, 1)

        @block.scalar
        def _(scalar: bass.BassEngine):
            if dual_ring:
                issue_phase_dmas(scalar, 0, (2, 3))
                if NPH > 1:
                    issue_phase_dmas(scalar, 1, (2, 3))
            # per phase: repack this phase, then issue phase ph+2's DMAs
            # (ph+1 already issued) so repack(0) is never starved
            r0, r1 = ROWS["act"]
            for ph in range(NPH):
                scalar.wait_ge(st_sems[ph], 16 * 8)
                if ph >= n_block:
                    scalar.wait_ge(pe_sem, ph - n_block + 1)
                scalar.activation(
                    blk4(ph % n_block, Gs[ph])[:, r0:r1, :, :],
                    stg4(ph % n_stage, Gs[ph])[:, r0:r1, :, :],
                    AFT.Copy).then_inc(rpk_act, 1)
                if dual_ring and ph + 2 < NPH:
                    if ph + 2 >= n_stage:
                        scalar.wait_ge(rpk_dve, ph + 2 - n_stage + 1)
                        scalar.wait_ge(rpk_act, ph + 2 - n_stage + 1)
                    issue_phase_dmas(scalar, ph + 2, (2, 3))

        @block.vector
        def _(vector: bass.BassEngine):
            r0, r1 = ROWS["dve"]
            for ph in range(NPH):
                vector.wait_ge(st_sems[ph], 16 * 8)
                if ph >= n_block:
                    vector.wait_ge(pe_sem, ph - n_block + 1)
                vector.tensor_copy(
                    blk4(ph % n_block, Gs[ph])[:, r0:r1, :, :],
                    stg4(ph % n_stage, Gs[ph])[:, r0:r1, :, :],
                ).then_inc(rpk_dve, 1)
            vector.wait_ge(pe_sem, NPH)
            vector.tensor_copy(out_sb, ps_g).then_inc(dve_sem, 1)

        @block.tensor
        def _(tensor: bass.BassEngine):
            nmm = 0
            total_mm = sum(Gs)
            for ph in range(NPH):
                v = ph + 1
                tensor.wait_ge(rpk_dve, v)
                tensor.wait_ge(rpk_act, v)
                tensor.wait_ge(rpk_gp, n_block)
                b = blk[ph % n_block]
                for g in range(Gs[ph]):
                    ap = b[:, BLK * g: BLK * g + 125]
                    mm = tensor.matmul(ps_g, ap, ap,
                                       start=(nmm == 0), stop=(nmm == total_mm - 1))
                    nmm += 1
                mm.then_inc(pe_sem, 1)

    meta = dict(T=T, widths=widths, dual_ring=dual_ring)
    return nc, meta


# ---------------- host side ----------------

EPS = 1e-8
LAM_SISNR, LAM_DIAR, LAM_EXIST = 1.0, 0.2, 0.2
TSUB = 1000
from itertools import permutations
PERMS = np.array(list(permutations(range(C))), dtype=np.int64)  # [6, 3]


def host_gamma_fp32(g125):
    """g125 [125,125] -> Gamma [25,25]; m = 5*r + c."""
    return np.einsum('acbc->ab', g125.reshape(25, 5, 25, 5).astype(np.float64))


def _clog(x):
    with np.errstate(divide='ignore'):
        return np.maximum(np.log(x), -100.0)


def host_finalize(gammas, exist_probs, num_speakers, diar_probs, labels,
                  T=128000):
    """gammas: list of [25,25] float64 per core. Returns the 5 scalars."""
    B = len(gammas) * B_LOC
    ns = np.asarray(num_speakers).astype(np.int64)

    S = np.zeros((B, C, C), np.float64)
    for core, gam in enumerate(gammas):
        for s in range(B_LOC):
            b = core * B_LOC + s
            e_rows = [s * 6 + i for i in range(3)]
            t_rows = [s * 6 + 3 + j for j in range(3)]
            dot_raw = gam[np.ix_(e_rows, t_rows)]            # [i, j]
            sep_sq = np.array([gam[r, r] for r in e_rows])
            src_sq = np.array([gam[r, r] for r in t_rows])
            sum_sep = gam[e_rows, 24]
            sum_src = gam[t_rows, 24]

            dot = dot_raw - np.outer(sum_sep, sum_src) / T
            est_sq = sep_sq - sum_sep ** 2 / T               # [i]
            tgt_sq = src_sq - sum_src ** 2 / T               # [j]

            alpha = dot / (tgt_sq[None, :] + EPS)
            sig = alpha * alpha * tgt_sq[None, :] + EPS
            noise = est_sq[:, None] - 2.0 * alpha * dot + alpha * alpha * tgt_sq[None, :] + EPS
            S[b] = 10.0 * np.log10(sig / noise)

    # ---- diarization BCE on host (tiny: B x 1000 x 3) ----
    dp = np.asarray(diar_probs, np.float64)
    lb = np.asarray(labels, np.float64)
    T_f = lb.shape[1]
    idx = np.floor(np.arange(TSUB) * (T_f / TSUB)).astype(np.int64)
    tgt_d = lb[:, idx, :]
    logp = _clog(dp)
    log1mp = _clog(1.0 - dp)
    D = -(np.einsum('bti,btj->bij', logp, tgt_d)
          + np.einsum('bti,btj->bij', log1mp, 1.0 - tgt_d)) / TSUB

    n_spk = np.clip(ns, 1, C)
    slot = np.arange(C)
    slot_mask = (slot[None, :] < n_spk[:, None]).astype(np.float64)
    valid = np.all((PERMS[None, :, :] < n_spk[:, None, None])
                   | (slot[None, None, :] >= n_spk[:, None, None]), axis=-1)

    S_perm = S[:, PERMS, slot]                               # [B, 6, 3]
    sisnr_mean = (S_perm * slot_mask[:, None, :]).sum(-1) / n_spk[:, None]
    sisnr_loss_p = np.where(valid, -sisnr_mean, np.inf)
    best = sisnr_loss_p.min(axis=-1)
    loss_sisnr = best.mean()
    mean_sisnr = (-best).mean()

    D_perm = D[:, PERMS, slot]
    diar_p = (D_perm * slot_mask[:, None, :]).sum(-1) / n_spk[:, None]
    loss_diar = np.where(valid, diar_p, np.inf).min(axis=-1).mean()

    ep = np.asarray(exist_probs, np.float64)
    n_ex = np.minimum(ns, C)
    ex_tgt = (np.arange(C + 1)[None, :] < n_ex[:, None]).astype(np.float64)
    bce_ex = -(ex_tgt * _clog(ep) + (1.0 - ex_tgt) * _clog(1.0 - ep))
    loss_exist = bce_ex.mean()

    total = LAM_SISNR * loss_sisnr + LAM_DIAR * loss_diar + LAM_EXIST * loss_exist
    return tuple(np.float32(v) for v in
                 (total, loss_sisnr, loss_diar, loss_exist, mean_sisnr))


def shard_inputs(separated, diar_probs, sources, labels, n_cores=8):
    maps = []
    for c in range(n_cores):
        sl = slice(B_LOC * c, B_LOC * (c + 1))
        maps.append({
            "sep": np.ascontiguousarray(separated[sl], dtype=np.float32),
            "src": np.ascontiguousarray(sources[sl], dtype=np.float32),
        })
    return maps


# ---------------- kernel entry (self-contained) ----------------

N_CORES = 8
_CACHE = {}


def _get_nc():
    if "nc" not in _CACHE:
        _CACHE["nc"] = build_nc(T=128000)[0]
    return _CACHE["nc"]


def kernel(separated, diar_probs, exist_probs, sources, labels, num_speakers):
    """EEND-SS loss on 8 NeuronCores: batch sharded 4 samples/core; device
    computes the big time-axis Gram; host does the tiny PIT/diar/exist math."""
    from concourse.bass_utils import run_bass_kernel_spmd

    separated = np.asarray(separated)
    diar_probs = np.asarray(diar_probs)
    exist_probs = np.asarray(exist_probs)
    sources = np.asarray(sources)
    labels = np.asarray(labels)
    num_speakers = np.asarray(num_speakers)

    nc = _get_nc()
    in_maps = shard_inputs(separated, diar_probs, sources, labels, N_CORES)
    res = run_bass_kernel_spmd(nc, in_maps, list(range(N_CORES)))

    gammas = [host_gamma_fp32(res.results[c]["gram"]) for c in range(N_CORES)]
    return host_finalize(gammas, exist_probs, num_speakers, diar_probs, labels,
                         T=128000)


# revision 9
# speedup vs baseline: 1.1877x; 1.0231x over previous
"""EEND-SS loss device kernel (raw Bass, SPMD over 8 cores).

Device computes, per core (B_LOC=4 samples), the Gram matrix of
[sep rows(12) | src rows(12) | ones] over T, chunk-blocked so the host
extracts all pairwise dots / sums / sq-sums for the SI-SDR PIT loss.
sep+src are 97% of the input bytes (12.3 MB/core); the tiny diarization
BCE (diar_probs 48 KB + labels 384 KB per 4 samples) and existence BCE
are done host-side in numpy -- their scattered-granule DMAs would grind
the SDMA engines for ~30us, longer than the compute they feed.

Pipeline per phase (T split into column-phases of widths[ph] chunks each):
  DMA (sequential HBM)        ->  staging[ph%4]   [128, 24*W] f32
  repack (DVE/ACT 17/7 rows)  ->  blocked[ph%5]   [128, 26*W] bf16
  PE matmuls on blocked (lhsT must be a single contiguous free dim)

Phase ph covers the contiguous T-range [128*offs[ph], 128*(offs[ph]+W)),
split contiguously by partition: partition p holds elems
[128*offs + p*W, 128*offs + (p+1)*W).  Consecutive descriptors then read
sequential HBM addresses (full row-buffer locality, ~25 GB/s/engine)
instead of 900B-granule hops on a 4KB stride (~19 GB/s/engine).  The
Gram is invariant to this time-permutation: all 24 rows + the ones row
of a sample use the same mapping, so every product pairs equal t.

Phase widths taper (..., 50): the tail after the last input byte is
last-phase repack + matmuls + output DMA, and scales with widths[-1].
Desc-gen (DIRECT2D ~0.7-1.3us per phase-DMA, serialized per HWDGE ring)
scales with len(widths) -- 5 phases keeps it well under the DMA window.

Blocked layout: free = (g, r, c): index = 130*g + 5*r + c, r in 0..25
 (rows 0..23 = data row r = s*6 + t*3 + i, row 24 = ones, row 25 = pad),
 c in 0..4, chunk = 5*g + c.

  per block g one matmul, lhsT = rhs = blocked[:, 130g:130g+125]
      psum_gram[125,125] += lhsT.T @ rhs ; m = 5*r + c
      host: Gamma[ra, rb] = sum_c psum[5*ra+c, 5*rb+c]
"""

import numpy as np
from contextlib import ExitStack

import concourse.bass as bass
from concourse import mybir

F32 = mybir.dt.float32
BF16 = mybir.dt.bfloat16
AFT = mybir.ActivationFunctionType

C = 3
B_LOC = 4
P = 128
PD = 125           # psum partition count (25 rows x 5 chunk-cols)
NROW = 26          # 24 data rows + ones(24) + pad(25)
NDATA = 24
BLK = 5 * NROW     # 130: free elems per chunk-group block
N_STAGE = 4
N_BLOCK = 5


def build_nc(T=128000, widths=(260, 260, 260, 160, 60), dual_ring=True):
    """Build the per-core Bass program. Returns (nc, meta)."""
    TCOLS = T // P
    assert TCOLS * P == T
    widths = tuple(widths)
    NPH = len(widths)
    assert sum(widths) == TCOLS
    assert all(w % 5 == 0 for w in widths)
    offs = [sum(widths[:i]) for i in range(NPH)]
    W_MAX = max(widths)
    Gs = [w // 5 for w in widths]   # blocks per phase
    n_stage = min(N_STAGE, NPH)
    n_block = min(N_BLOCK, NPH)

    nc = bass.Bass(trn_type="TRN2", target_bir_lowering=False, debug=False)

    sep = nc.dram_tensor("sep", [B_LOC, C, T], F32, kind="ExternalInput").ap()
    src = nc.dram_tensor("src", [B_LOC, C, T], F32, kind="ExternalInput").ap()
    gram_out = nc.dram_tensor("gram", [PD, PD], F32, kind="ExternalOutput").ap()

    # SBUF
    stg = [nc.alloc_sbuf_tensor(f"stg{i}", [P, NDATA * W_MAX], F32).ap()
           for i in range(n_stage)]
    blk = [nc.alloc_sbuf_tensor(f"blk{i}", [P, NROW * W_MAX], BF16).ap()
           for i in range(n_block)]
    out_sb = nc.alloc_sbuf_tensor("out_sb", [PD, PD], F32).ap()

    # PSUM
    ps_g = nc.alloc_psum_tensor("ps_g", [PD, PD], F32).ap()

    # helper views (per-phase W: use the buffer prefix so rows stay packed)
    def stg3(i, w):
        return stg[i][:, :NDATA * w].rearrange("p (r n) -> p r n", r=NDATA)

    def stg4(i, g):  # [p, r, g, c]
        return stg[i][:, :NDATA * g * 5].rearrange(
            "p (r g c) -> p r g c", r=NDATA, g=g)

    def blk4(i, g):  # [p, r, g, c] view of blocked (g, r, c) layout
        return blk[i][:, :NROW * g * 5].rearrange(
            "p (g r c) -> p r g c", g=g, r=NROW)

    # repack row split across the two fast copy engines: while desc-gen still
    # occupies ACT (early phases) give DVE more rows; in the tail split evenly
    def rows_of(ph):
        return ((0, 12), (12, 24)) if ph >= NPH - 2 else ((0, 17), (17, 24))

    with ExitStack() as ctx:
        # one counting sem per HWDGE ring: completion is FIFO per ring, so
        # ring_sem >= 64*(ph+1) iff all of phases 0..ph landed on that ring
        ring_sems = [ctx.enter_context(nc.semaphore(f"ring_sem{i}"))
                     for i in range(2 if dual_ring else 1)]
        odma_sem = ctx.enter_context(nc.semaphore("odma_sem"))
        rpk_dve = ctx.enter_context(nc.semaphore("rpk_dve"))
        rpk_act = ctx.enter_context(nc.semaphore("rpk_act"))
        rpk_gp = ctx.enter_context(nc.semaphore("rpk_gp"))
        dve_sem = ctx.enter_context(nc.semaphore("dve_sem"))
        pe_sem = ctx.enter_context(nc.semaphore("pe_sem"))
        block = ctx.enter_context(nc.Block())

        per_ring = (2 if dual_ring else 4) * 2 * 16  # sem incs per phase per ring

        def issue_phase_dmas(eng, ph, samples, ring):
            s3 = stg3(ph % n_stage, widths[ph])
            w = widths[ph]
            for s in samples:
                for t, big in enumerate((sep, src)):
                    r0 = s * 6 + t * 3
                    src_ap = big[s][:, P * offs[ph]:P * (offs[ph] + w)
                                    ].rearrange("i (p w) -> p i w", p=P)
                    eng.dma_start(out=s3[:, r0:r0 + 3, :], in_=src_ap
                                  ).then_inc(ring_sems[ring], 16)

        def wait_phase_data(eng, ph):
            for rs in ring_sems:
                eng.wait_ge(rs, per_ring * (ph + 1))

        @block.sync
        def _(sync: bass.BassEngine):
            # big data split across both HWDGE rings (desc-gen is the issue-rate
            # bottleneck: ~0.7-1.3us per DMA, serialized per ring)
            for ph in range(NPH):
                if ph >= n_stage:
                    # WAR: staging slot reused; wait repack of ph - n_stage done
                    v = ph - n_stage + 1
                    sync.wait_ge(rpk_dve, v)
                    sync.wait_ge(rpk_act, v)
                issue_phase_dmas(sync, ph, (0, 1) if dual_ring else (0, 1, 2, 3),
                                 ring=0)

            # output at the very end
            sync.wait_ge(dve_sem, 1)
            sync.dma_start(out=gram_out, in_=out_sb).then_inc(odma_sem, 16)
            sync.wait_ge(odma_sem, 16)

        @block.gpsimd
        def _(gpsimd: bass.BassEngine):
            # ones(row 24) + pad(row 25) once per blocked slot; no repack here
            # (gpsimd copies measured ~6x slower than DVE)
            for i in range(n_block):
                ap1 = blk[i].rearrange("p (g x) -> p g x", g=W_MAX // 5)[
                    :, :, 5 * NDATA:5 * NROW]
                gpsimd.memset(ap1, 1.0).then_inc(rpk_gp, 1)

        @block.scalar
        def _(scalar: bass.BassEngine):
            if dual_ring:
                issue_phase_dmas(scalar, 0, (2, 3), ring=1)
                if NPH > 1:
                    issue_phase_dmas(scalar, 1, (2, 3), ring=1)
            # per phase: repack this phase, then issue phase ph+2's DMAs
            # (ph+1 already issued) so repack(0) is never starved
            for ph in range(NPH):
                r0, r1 = rows_of(ph)[1]
                wait_phase_data(scalar, ph)
                if ph >= n_block:
                    scalar.wait_ge(pe_sem, ph - n_block + 1)
                scalar.activation(
                    blk4(ph % n_block, Gs[ph])[:, r0:r1, :, :],
                    stg4(ph % n_stage, Gs[ph])[:, r0:r1, :, :],
                    AFT.Copy).then_inc(rpk_act, 1)
                if dual_ring and ph + 2 < NPH:
                    if ph + 2 >= n_stage:
                        scalar.wait_ge(rpk_dve, ph + 2 - n_stage + 1)
                        scalar.wait_ge(rpk_act, ph + 2 - n_stage + 1)
                    issue_phase_dmas(scalar, ph + 2, (2, 3), ring=1)

        @block.vector
        def _(vector: bass.BassEngine):
            for ph in range(NPH):
                r0, r1 = rows_of(ph)[0]
                wait_phase_data(vector, ph)
                if ph >= n_block:
                    vector.wait_ge(pe_sem, ph - n_block + 1)
                vector.tensor_copy(
                    blk4(ph % n_block, Gs[ph])[:, r0:r1, :, :],
                    stg4(ph % n_stage, Gs[ph])[:, r0:r1, :, :],
                ).then_inc(rpk_dve, 1)
            vector.wait_ge(pe_sem, NPH)
            vector.tensor_copy(out_sb, ps_g).then_inc(dve_sem, 1)

        @block.tensor
        def _(tensor: bass.BassEngine):
            nmm = 0
            total_mm = sum(Gs)
            for ph in range(NPH):
                v = ph + 1
                tensor.wait_ge(rpk_dve, v)
                tensor.wait_ge(rpk_act, v)
                tensor.wait_ge(rpk_gp, n_block)
                b = blk[ph % n_block]
                for g in range(Gs[ph]):
                    ap = b[:, BLK * g: BLK * g + 125]
                    mm = tensor.matmul(ps_g, ap, ap,
                                       start=(nmm == 0), stop=(nmm == total_mm - 1))
                    nmm += 1
                mm.then_inc(pe_sem, 1)

    meta = dict(T=T, widths=widths, dual_ring=dual_ring)
    return nc, meta


# ---------------- host side ----------------

EPS = 1e-8
LAM_SISNR, LAM_DIAR, LAM_EXIST = 1.0, 0.2, 0.2
TSUB = 1000
from itertools import permutations
PERMS = np.array(list(permutations(range(C))), dtype=np.int64)  # [6, 3]


def host_gamma_fp32(g125):
    """g125 [125,125] -> Gamma [25,25]; m = 5*r + c."""
    return np.einsum('acbc->ab', g125.reshape(25, 5, 25, 5).astype(np.float64))


def _clog(x):
    with np.errstate(divide='ignore'):
        return np.maximum(np.log(x), -100.0)


def host_finalize(gammas, exist_probs, num_speakers, diar_probs, labels,
                  T=128000):
    """gammas: list of [25,25] float64 per core. Returns the 5 scalars."""
    B = len(gammas) * B_LOC
    ns = np.asarray(num_speakers).astype(np.int64)

    S = np.zeros((B, C, C), np.float64)
    for core, gam in enumerate(gammas):
        for s in range(B_LOC):
            b = core * B_LOC + s
            e_rows = [s * 6 + i for i in range(3)]
            t_rows = [s * 6 + 3 + j for j in range(3)]
            dot_raw = gam[np.ix_(e_rows, t_rows)]            # [i, j]
            sep_sq = np.array([gam[r, r] for r in e_rows])
            src_sq = np.array([gam[r, r] for r in t_rows])
            sum_sep = gam[e_rows, 24]
            sum_src = gam[t_rows, 24]

            dot = dot_raw - np.outer(sum_sep, sum_src) / T
            est_sq = sep_sq - sum_sep ** 2 / T               # [i]
            tgt_sq = src_sq - sum_src ** 2 / T               # [j]

            alpha = dot / (tgt_sq[None, :] + EPS)
            sig = alpha * alpha * tgt_sq[None, :] + EPS
            noise = est_sq[:, None] - 2.0 * alpha * dot + alpha * alpha * tgt_sq[None, :] + EPS
            S[b] = 10.0 * np.log10(sig / noise)

    # ---- diarization BCE on host (tiny: B x 1000 x 3) ----
    dp = np.asarray(diar_probs, np.float64)
    lb = np.asarray(labels, np.float64)
    T_f = lb.shape[1]
    idx = np.floor(np.arange(TSUB) * (T_f / TSUB)).astype(np.int64)
    tgt_d = lb[:, idx, :]
    logp = _clog(dp)
    log1mp = _clog(1.0 - dp)
    D = -(np.einsum('bti,btj->bij', logp, tgt_d)
          + np.einsum('bti,btj->bij', log1mp, 1.0 - tgt_d)) / TSUB

    n_spk = np.clip(ns, 1, C)
    slot = np.arange(C)
    slot_mask = (slot[None, :] < n_spk[:, None]).astype(np.float64)
    valid = np.all((PERMS[None, :, :] < n_spk[:, None, None])
                   | (slot[None, None, :] >= n_spk[:, None, None]), axis=-1)

    S_perm = S[:, PERMS, slot]                               # [B, 6, 3]
    sisnr_mean = (S_perm * slot_mask[:, None, :]).sum(-1) / n_spk[:, None]
    sisnr_loss_p = np.where(valid, -sisnr_mean, np.inf)
    best = sisnr_loss_p.min(axis=-1)
    loss_sisnr = best.mean()
    mean_sisnr = (-best).mean()

    D_perm = D[:, PERMS, slot]
    diar_p = (D_perm * slot_mask[:, None, :]).sum(-1) / n_spk[:, None]
    loss_diar = np.where(valid, diar_p, np.inf).min(axis=-1).mean()

    ep = np.asarray(exist_probs, np.float64)
    n_ex = np.minimum(ns, C)
    ex_tgt = (np.arange(C + 1)[None, :] < n_ex[:, None]).astype(np.float64)
    bce_ex = -(ex_tgt * _clog(ep) + (1.0 - ex_tgt) * _clog(1.0 - ep))
    loss_exist = bce_ex.mean()

    total = LAM_SISNR * loss_sisnr + LAM_DIAR * loss_diar + LAM_EXIST * loss_exist
    return tuple(np.float32(v) for v in
                 (total, loss_sisnr, loss_diar, loss_exist, mean_sisnr))


def shard_inputs(separated, diar_probs, sources, labels, n_cores=8):
    maps = []
    for c in range(n_cores):
        sl = slice(B_LOC * c, B_LOC * (c + 1))
        maps.append({
            "sep": np.ascontiguousarray(separated[sl], dtype=np.float32),
            "src": np.ascontiguousarray(sources[sl], dtype=np.float32),
        })
    return maps


# ---------------- kernel entry (self-contained) ----------------

N_CORES = 8
_CACHE = {}


def _get_nc():
    if "nc" not in _CACHE:
        _CACHE["nc"] = build_nc(T=128000)[0]
    return _CACHE["nc"]


def kernel(separated, diar_probs, exist_probs, sources, labels, num_speakers):
    """EEND-SS loss on 8 NeuronCores: batch sharded 4 samples/core; device
    computes the big time-axis Gram; host does the tiny PIT/diar/exist math."""
    from concourse.bass_utils import run_bass_kernel_spmd

    separated = np.asarray(separated)
    diar_probs = np.asarray(diar_probs)
    exist_probs = np.asarray(exist_probs)
    sources = np.asarray(sources)
    labels = np.asarray(labels)
    num_speakers = np.asarray(num_speakers)

    nc = _get_nc()
    in_maps = shard_inputs(separated, diar_probs, sources, labels, N_CORES)
    res = run_bass_kernel_spmd(nc, in_maps, list(range(N_CORES)))

    gammas = [host_gamma_fp32(res.results[c]["gram"]) for c in range(N_CORES)]
    return host_finalize(gammas, exist_probs, num_speakers, diar_probs, labels,
                         T=128000)
